# revision 32
# baseline (speedup 1.0000x reference)
"""Kent-distribution pairwise KLD loss kernel for Trainium2 (8 NeuronCores).

The [N, M] pairwise KLD matrix factors exactly as a rank-11 product
U @ V^T:

  KLD[n, m] = A[n]                                  (pred-row constant)
            + c_b'[m]                               (target-row constant)
            - Ex_a[n] . (kappa_b[m] * gamma_b1[m])  (rank 3)
            + <ExxT_a[n], beta_b[m]*(g_b3 g_b3^T - g_b2 g_b2^T)>  (rank 6, sym)

so each core computes 11 features per pred row (its N-shard) and per
target row (replicated), then one skinny fp16 matmul [256,11]@[11,2048]
accumulated in fp32 PSUM.  N is sharded across the 8 cores.

Numerics notes (validated vs the jax reference):
 - exp(c_k - c), exp(c_kk - c) are evaluated as exact algebraic ratios:
   l1 = (D - k)/D and l2 = (2kD - 2k^2 - s)/(2 D^2), D = k^2 - s,
   s = 4 b^2.  exp(c_beta - c) carries e^-kappa <= 4.5e-5 and is
   dropped; with lambda2 == lambda3, ExxT = l2 I + (l1-l2) g1 g1^T.
 - ln(2pi) cancels between c_b and c_a, so it is dropped from both.
 - |gamma_a1|^2 == 1 exactly, so A = 0.5 ln(prod_a) - k + k*l1.
 - features are rounded to fp16 for the PE (rank-11 dot, fp32 PSUM
   accumulate): adds ~1e-3 max-rel error, far inside the 2e-2 gate.
 - Sin's HW domain is [-pi, pi]: cos(x) = sin(pi/2 - |x|).
"""

import sys

import numpy as np

sys.path.insert(0, "/opt/trn_rl_repo")

import concourse.bass as bass  # noqa: E402,F401
import concourse.mybir as mybir  # noqa: E402
import concourse.tile as tile  # noqa: E402
from concourse import bacc  # noqa: E402
from concourse.masks import make_identity  # noqa: E402

F32 = mybir.dt.float32
F16 = mybir.dt.float16
AF = mybir.ActivationFunctionType
ALU = mybir.AluOpType

N = 2048
M = 2048
NCORES = 8
NS = N // NCORES  # 256 pred rows per core
K = 11  # feature rank
GP = NS // 128  # pred row-groups (2)
GT = M // 128  # target row-groups (16)
G = GP + GT  # 18

PI = float(np.pi)
EPS = 1e-6


def _body(tc, cat, out):
    nc = tc.nc
    with (
        tc.tile_pool(name="main", bufs=1) as pool,
        tc.tile_pool(name="tp_psum", bufs=2, space="PSUM") as tpp,
        tc.tile_pool(name="ut_psum", bufs=1, space="PSUM") as upp,
        tc.tile_pool(name="warm_psum", bufs=1, space="PSUM") as wpp,
        tc.tile_pool(name="out_psum", bufs=4, space="PSUM") as opp,
    ):
        def t(shape, tag, dtype=F32):
            return pool.tile([128, *shape], dtype, name=tag, tag=tag)

        # ---- constants + identity (run in the preamble shadow)
        half_pi = pool.tile([128, 1], F32, name="half_pi", tag="half_pi")
        nc.gpsimd.memset(half_pi, PI / 2)
        eps_c = pool.tile([128, 1], F32, name="eps_c", tag="eps_c")
        nc.gpsimd.memset(eps_c, EPS)
        ident = pool.tile([128, 128], F16, name="ident", tag="ident")
        make_identity(nc, ident)
        # dummy Sin + dummy Ln on a constant hoist BOTH ACT_TABLE_LOADs
        # (2 x 1283ns) into the preamble/input-DMA shadow; the two table sets
        # occupy different slots so both stay resident
        sin_dummy = pool.tile([128, 1], F32, name="sin_dummy", tag="sin_dummy")
        nc.scalar.activation(sin_dummy[:], half_pi[:], AF.Sin)
        nc.scalar.activation(sin_dummy[:], half_pi[:], AF.Ln)
        # PE p-state warmup: the PE clock ramps 0.65->2.4 GHz only under
        # sustained work; idle until the transposes means everything runs at
        # the slow clock.  Burn the input-DMA/feature wait on dummy
        # transposes so the real transposes + matmuls hit the fast clock.
        # K=1 matmuls: ~430ns of PE busy each but only ~1KB of SBUF reads, so
        # they don't steal SBUF bandwidth from the DVE/Pool feature ops.
        N_WARM = 11
        if N_WARM:
            warm = wpp.tile([128, 512], F32, name="warm", tag="warm")
            wrow = ident[0:1, :]
            wmov = wrow.unsqueeze(1).broadcast_to([1, 4, 128])
            for _ in range(N_WARM):
                nc.tensor.matmul(warm[:], wrow, wmov, start=True, stop=True)

        # ---- single input DMA: host pre-interleaves pred|targ rows so
        # partition p holds pred rows 2p,2p+1 then targ rows 16p..16p+15.
        params = t([G * 5], "params")
        nc.sync.dma_start(out=params[:], in_=cat)

        P5 = params.rearrange("p (g c) -> p c g", c=5)  # [128, 5, 18]
        kap = P5[:, 3, :]  # [128, 18] stride-5 slabs
        bet = P5[:, 4, :]
        angles = P5[:, 0:3, :]  # eta, alpha, psi
        kap_p, kap_t = kap[:, 0:GP], kap[:, GP:G]
        bet_t = bet[:, GP:G]
        TT = slice(GP, G)

        # feature tiles (fp16); constant rows preset in the preamble shadow
        VF = t([K, GT], "VF", F16)
        nc.gpsimd.memset(VF[:, 0, :], 1.0)
        UF = t([K, GP], "UF", F16)
        nc.gpsimd.memset(UF[:, 1, :], 1.0)

        # tiles
        absv = t([3, G], "absv")
        trig = t([6, G], "trig")
        x2 = t([G], "x2")
        s4 = t([G], "s4")
        D = t([G], "D")
        LNOUT = t([G], "LNOUT")
        pp4 = t([2, 2, G], "pp4")
        m24 = t([2, G], "m24")
        mm4 = t([2, 2, G], "mm4")
        g23 = t([2, 3, GT], "g23")
        p23 = t([2, 6, GT], "p23")
        dV = t([6, GT], "dV")
        gam1 = t([3, G], "gam1")
        ppd = t([3, GP], "ppd")
        ppo = t([3, GP], "ppo")
        r = t([GP], "r")
        neg = t([GP], "neg")
        l1 = t([GP], "l1")
        kD = t([GP], "kD")
        Q = t([GP], "Q")
        r2 = t([GP], "r2")
        l2 = t([GP], "l2")
        dE = t([GP], "dE")
        edt = t([3, GP], "edt")
        kad = t([GP], "kad")
        a1 = t([GP], "a1")

        ce, ca, cp = trig[:, 0, :], trig[:, 1, :], trig[:, 2, :]
        se, sa, sp = trig[:, 3, :], trig[:, 4, :], trig[:, 5, :]
        T6 = trig.rearrange("p (s a) g -> p s a g", a=3)  # s: cos/sin, a: angle
        cpsp = T6[:, :, 2, :]  # rows {cp, sp}  [128, 2, 18]
        cese = T6[:, :, 0, :]  # rows {ce, se}  [128, 2, 18]
        m2ce, m2se = mm4[:, 0, 0, :], mm4[:, 0, 1, :]
        m4ce, m4se = mm4[:, 1, 0, :], mm4[:, 1, 1, :]
        cpce, cpse = pp4[:, 0, 0, :], pp4[:, 0, 1, :]
        spce, spse = pp4[:, 1, 0, :], pp4[:, 1, 1, :]
        g1p = gam1[:, :, 0:GP]  # [128, 3, 2]
        D_p = D[:, 0:GP]

        # =====================================================================
        # Feature computation.  IMPORTANT: the tile framework is sequentially
        # consistent with Python ISSUE order -- a read issued before the
        # producing write sees the OLD tile contents.  Everything below is in
        # topological order; engine choice (V=DVE, P=Pool, ACT) balances load.
        # =====================================================================
        V = nc.vector
        P = nc.gpsimd

        # |angles| feeds ACT cos; trig rows: 0..2 = cos(e,a,p), 3..5 = sin
        V.scalar_tensor_tensor(absv[:], angles, -1.0, angles, ALU.mult, ALU.max)
        nc.scalar.activation(trig[:, 3:6, :], angles, AF.Sin)
        nc.scalar.activation(trig[:, 0:3, :], absv[:], AF.Sin,
                             bias=half_pi, scale=-1.0)
        # c-denominator for all 18 groups; Ln table load overlaps DVE/Pool
        V.tensor_mul(x2, kap, kap)                                   # k^2
        V.scalar_tensor_tensor(s4, bet, 4.0, bet, ALU.mult, ALU.mult)  # 4b^2
        V.tensor_sub(D, x2, s4)                                      # k^2-4b^2
        nc.scalar.activation(LNOUT[:], D[:], AF.Ln, bias=eps_c)
        # trig products (Pool m24->mm4 chain runs parallel to DVE pp4)
        P.tensor_mul(m24, cpsp, ca.unsqueeze(1).broadcast_to([128, 2, G]))
        V.tensor_mul(
            pp4,
            cpsp.unsqueeze(2).broadcast_to([128, 2, 2, G]),
            cese.unsqueeze(1).broadcast_to([128, 2, 2, G]),
        )
        P.tensor_mul(
            mm4,
            m24.unsqueeze(2).broadcast_to([128, 2, 2, G]),
            cese.unsqueeze(1).broadcast_to([128, 2, 2, G]),
        )
        P.tensor_copy(gam1[:, 0, :], ca)
        P.tensor_mul(gam1[:, 1:3, :], cese,
                     sa.unsqueeze(1).broadcast_to([128, 2, G]))
        V.scalar_tensor_tensor(g23[:, 0, 0, :], cp[:, TT], -1.0, sa[:, TT],
                               ALU.mult, ALU.mult)                   # -cp*sa
        # pred lambda chain: l1 = (D-k)/D, l2 = (2k(D-k) - s)/(2D^2)
        V.reciprocal(r, D_p)
        V.tensor_sub(neg, D_p, kap_p)
        V.tensor_mul(l1, neg, r)
        V.tensor_mul(kD, kap_p, neg)                                 # k(D-k)
        V.scalar_tensor_tensor(Q, kD, 2.0, s4[:, 0:GP],
                               ALU.mult, ALU.subtract)               # 2k(D-k)-s
        V.tensor_mul(r2, r, r)
        V.scalar_tensor_tensor(l2, Q, 0.5, r2, ALU.mult, ALU.mult)
        V.tensor_sub(dE, l1, l2)
        # gamma2 / gamma3 components (targets only)
        P.tensor_sub(g23[:, 0, 1, :], m2ce[:, TT], spse[:, TT])
        P.tensor_add(g23[:, 0, 2, :], m2se[:, TT], spce[:, TT])
        P.tensor_mul(g23[:, 1, 0, :], sp[:, TT], sa[:, TT])
        V.scalar_tensor_tensor(g23[:, 1, 1, :], m4ce[:, TT], -1.0,
                               cpse[:, TT], ALU.mult, ALU.subtract)  # g3_1
        P.tensor_sub(g23[:, 1, 2, :], cpce[:, TT], m4se[:, TT])
        # pred g1 pair products: diag squares on ACT (its tables are already
        # loaded, so it is free after LNOUT), offdiags on Pool
        nc.scalar.activation(ppd[:], g1p, AF.Square)
        P.tensor_mul(
            ppo[:, 0:2, :],
            gam1[:, 0:1, 0:GP].broadcast_to([128, 2, GP]),
            gam1[:, 1:3, 0:GP],
        )
        P.tensor_mul(ppo[:, 2, :], gam1[:, 1, 0:GP], gam1[:, 2, 0:GP])
        # target pair products: squares on ACT, offdiags on Pool
        nc.scalar.activation(p23[:, :, 0:3, :], g23[:], AF.Square)
        P.tensor_mul(
            p23[:, :, 3:5, :],
            g23[:, :, 0:1, :].broadcast_to([128, 2, 2, GT]),
            g23[:, :, 1:3, :],
        )
        P.tensor_mul(p23[:, :, 5, :], g23[:, :, 1, :], g23[:, :, 2, :])
        V.tensor_sub(dV, p23[:, 1, :, :], p23[:, 0, :, :])
        # VF build (fp16 out): rank 1 = c_b' = k - 0.5 ln(prod)  (ln2pi
        # cancels against c_a); 2-4 = -k*g1; 5-7 = b*dV_d; 8-10 = 2b*dV_o
        V.scalar_tensor_tensor(VF[:, 1, :], LNOUT[:, GP:G], -0.5, kap_t,
                               ALU.mult, ALU.add)
        V.scalar_tensor_tensor(
            VF[:, 2:5, :], gam1[:, :, GP:G], -1.0,
            kap_t.unsqueeze(1).broadcast_to([128, 3, GT]),
            ALU.mult, ALU.mult,
        )
        P.tensor_mul(VF[:, 5:8, :], dV[:, 0:3, :],
                     bet_t.unsqueeze(1).broadcast_to([128, 3, GT]))
        V.scalar_tensor_tensor(
            VF[:, 8:11, :], dV[:, 3:6, :], 2.0,
            bet_t.unsqueeze(1).broadcast_to([128, 3, GT]),
            ALU.mult, ALU.mult,
        )
        # UF build
        l1b = l1.unsqueeze(1).broadcast_to([128, 3, GP])
        dEb = dE.unsqueeze(1).broadcast_to([128, 3, GP])
        V.tensor_mul(UF[:, 2:5, :], g1p, l1b)
        V.tensor_mul(edt, ppd, dEb)
        V.tensor_add(UF[:, 5:8, :], edt,
                     l2.unsqueeze(1).broadcast_to([128, 3, GP]))
        V.tensor_mul(UF[:, 8:11, :], ppo, dEb)
        V.tensor_mul(kad, kap_p, l1)
        V.scalar_tensor_tensor(a1, LNOUT[:, 0:GP], 0.5, kap_p,
                               ALU.mult, ALU.subtract)
        V.tensor_add(UF[:, 0, :], a1, kad)

        # =====================================================================
        # PE: transposes (fp16) + matmuls, interleaved so matmuls start as VT
        # chunks land.  UT first (it gates every matmul).
        # =====================================================================
        # pred rows are loaded blocked (row 128t + p on partition p), so the
        # transposed feature block for group t is directly UT cols 128t..+128
        UT = pool.tile([K, NS], F16, name="UT", tag="UT")
        utp = upp.tile([K, GP * 128], F16, name="utp", tag="utp")
        for j in range(GP):
            nc.tensor.transpose(utp[:, j * 128 : (j + 1) * 128], UF[:, :, j],
                                ident[:])
        nc.scalar.copy(UT[:], utp[:])

        # PSUM can only be read by the scalar (ACT) and vector (DVE) engines
        VT = pool.tile([K, M], F16, name="VT", tag="VT")
        copy_engines = [nc.scalar, nc.vector]

        def vt_chunk(q):
            # two half-copies on both PSUM-capable engines: halves the
            # transpose-to-matmul latency vs one [11, 512] copy
            vtp = tpp.tile([K, 512], F16, name="vtp", tag="vtp", bufs=2)
            for jj in range(4):
                j = q * 4 + jj
                nc.tensor.transpose(
                    vtp[:, jj * 128 : (jj + 1) * 128], VF[:, :, j], ident[:]
                )
            nc.scalar.copy(
                VT[:, q * 512 : q * 512 + 256], vtp[:, 0:256]
            )
            nc.vector.tensor_copy(
                VT[:, q * 512 + 256 : (q + 1) * 512], vtp[:, 256:512]
            )

        # main matmuls: targets are loaded blocked (row 128j + p on partition
        # p), so VT col 128j + p = target row and psum col f of chunk c is
        # target row 512c + f -- each matmul depends on exactly one vt_chunk
        mm_i = 0

        def mm(ti, c):
            nonlocal mm_i
            ops = opp.tile([128, 512], F32, name="ops", tag="ops")
            nc.tensor.matmul(
                ops[:],
                UT[:, 128 * ti : 128 * (ti + 1)],
                VT[:, 512 * c : 512 * (c + 1)],
                start=True,
                stop=True,
            )
            out_sb = pool.tile([128, 512], F32, name="out_sb", tag="out_sb",
                               bufs=4)
            orow = out[128 * ti : 128 * (ti + 1), 512 * c : 512 * (c + 1)]
            if mm_i == 0:
                # fast-path the first block: halve the PSUM->SBUF copy across
                # both engines and issue its DMA immediately -- the whole
                # output drain is BW-bound from the FIRST packet onward
                nc.scalar.copy(out_sb[:, 0:256], ops[:, 0:256])
                nc.sync.dma_start(out=orow[:, 0:256], in_=out_sb[:, 0:256])
                nc.vector.tensor_copy(out_sb[:, 256:512], ops[:, 256:512])
                nc.sync.dma_start(out=orow[:, 256:512], in_=out_sb[:, 256:512])
            else:
                eng = copy_engines[mm_i % 2]
                if eng is nc.scalar:
                    nc.scalar.copy(out_sb[:], ops[:])
                else:
                    eng.tensor_copy(out_sb[:], ops[:])
                nc.sync.dma_start(out=orow, in_=out_sb[:])
            mm_i += 1

        for c in range(4):
            vt_chunk(c)
            mm(0, c)
            mm(1, c)


def build():
    nc = bacc.Bacc()
    cat = nc.dram_tensor("cat", [128, G * 5], F32, kind="ExternalInput")
    out = nc.dram_tensor("out", [NS, M], F32, kind="ExternalOutput")
    with tile.TileContext(nc) as tc:
        _body(tc, cat[:], out[:])
    nc.finalize()
    return nc


_NC_CACHE = None


def _get_nc():
    global _NC_CACHE
    if _NC_CACHE is None:
        _NC_CACHE = build()
    return _NC_CACHE


def kernel(kent_pred, kent_target, trace=False, tmpdir=None):
    from concourse.bass_utils import run_bass_kernel_spmd

    nc = _get_nc()
    kent_pred = np.ascontiguousarray(np.asarray(kent_pred, dtype=np.float32))
    kent_target = np.ascontiguousarray(np.asarray(kent_target, dtype=np.float32))
    # blocked layout: partition p holds pred rows {p, 128+p} (local) then
    # targ rows {128j + p} -- one contiguous [128, 90] buffer per core
    targ_il = np.ascontiguousarray(
        kent_target.reshape(GT, 128, 5).transpose(1, 0, 2).reshape(128, GT * 5)
    )
    in_maps = []
    for i in range(NCORES):
        pred_il = (
            kent_pred[i * NS : (i + 1) * NS]
            .reshape(GP, 128, 5)
            .transpose(1, 0, 2)
            .reshape(128, GP * 5)
        )
        in_maps.append(
            {"cat": np.ascontiguousarray(np.concatenate([pred_il, targ_il], axis=1))}
        )
    res = run_bass_kernel_spmd(
        nc, in_maps, core_ids=list(range(NCORES)), trace=trace, tmpdir=tmpdir
    )
    out = np.concatenate([r["out"] for r in res.results], axis=0)
    if trace:
        kernel.last_results = res
    return out


# revision 33
# speedup vs baseline: 1.0656x; 1.0656x over previous
"""Kent-distribution pairwise KLD loss kernel for Trainium2 (8 NeuronCores).

The [N, M] pairwise KLD matrix factors exactly as a rank-11 product
U @ V^T:

  KLD[n, m] = A[n]                                  (pred-row constant)
            + c_b'[m]                               (target-row constant)
            - Ex_a[n] . (kappa_b[m] * gamma_b1[m])  (rank 3)
            + <ExxT_a[n], beta_b[m]*(g_b3 g_b3^T - g_b2 g_b2^T)>  (rank 6, sym)

so each core computes 11 features per pred row (its N-shard) and per
target row (replicated), then one skinny fp16 matmul [256,11]@[11,2048]
accumulated in fp32 PSUM.  N is sharded across the 8 cores.

Numerics notes (validated vs the jax reference):
 - exp(c_k - c), exp(c_kk - c) are evaluated as exact algebraic ratios:
   l1 = (D - k)/D and l2 = (2kD - 2k^2 - s)/(2 D^2), D = k^2 - s,
   s = 4 b^2.  exp(c_beta - c) carries e^-kappa <= 4.5e-5 and is
   dropped; with lambda2 == lambda3, ExxT = l2 I + (l1-l2) g1 g1^T.
 - ln(2pi) cancels between c_b and c_a, so it is dropped from both.
 - |gamma_a1|^2 == 1 exactly, so A = 0.5 ln(prod_a) - k + k*l1.
 - features are rounded to fp16 for the PE (rank-11 dot, fp32 PSUM
   accumulate): adds ~1e-3 max-rel error, far inside the 2e-2 gate.
 - Sin's HW domain is [-pi, pi]: cos(x) = sin(pi/2 - |x|).
"""

import sys

import numpy as np

sys.path.insert(0, "/opt/trn_rl_repo")

import concourse.bass as bass  # noqa: E402,F401
import concourse.mybir as mybir  # noqa: E402
import concourse.tile as tile  # noqa: E402
from concourse import bacc  # noqa: E402
from concourse.masks import make_identity  # noqa: E402

F32 = mybir.dt.float32
F16 = mybir.dt.float16
AF = mybir.ActivationFunctionType
ALU = mybir.AluOpType

N = 2048
M = 2048
NCORES = 8
NS = N // NCORES  # 256 pred rows per core
K = 11  # feature rank
GP = NS // 128  # pred row-groups (2)
GT = M // 128  # target row-groups (16)
G = GP + GT  # 18

PI = float(np.pi)
EPS = 1e-6


def _body(tc, cat, out):
    nc = tc.nc
    with (
        tc.tile_pool(name="main", bufs=1) as pool,
        tc.tile_pool(name="tp_psum", bufs=2, space="PSUM") as tpp,
        tc.tile_pool(name="ut_psum", bufs=1, space="PSUM") as upp,
        tc.tile_pool(name="warm_psum", bufs=1, space="PSUM") as wpp,
        tc.tile_pool(name="out_psum", bufs=4, space="PSUM") as opp,
    ):
        def t(shape, tag, dtype=F32):
            return pool.tile([128, *shape], dtype, name=tag, tag=tag)

        # ---- constants + identity (run in the preamble shadow)
        half_pi = pool.tile([128, 1], F32, name="half_pi", tag="half_pi")
        nc.gpsimd.memset(half_pi, PI / 2)
        eps_c = pool.tile([128, 1], F32, name="eps_c", tag="eps_c")
        nc.gpsimd.memset(eps_c, EPS)
        ident = pool.tile([128, 128], F16, name="ident", tag="ident")
        make_identity(nc, ident)
        # dummy Sin on a constant hoists the trig ACT_TABLE_LOAD off the
        # input-DMA critical path.  NOTE: the compiler reloads the table on
        # EVERY function-set switch, so the ACT queue must stay grouped:
        # sin-set ops first, then ln-set ops (Ln/Square/Copy share a set).
        sin_dummy = pool.tile([128, 1], F32, name="sin_dummy", tag="sin_dummy")
        nc.scalar.activation(sin_dummy[:], half_pi[:], AF.Sin)
        # PE p-state warmup: the PE clock ramps 0.65->2.4 GHz only under
        # sustained work; idle until the transposes means everything runs at
        # the slow clock.  Burn the input-DMA/feature wait on dummy
        # transposes so the real transposes + matmuls hit the fast clock.
        # K=1 matmuls: ~430ns of PE busy each but only ~1KB of SBUF reads, so
        # they don't steal SBUF bandwidth from the DVE/Pool feature ops.
        N_WARM = 11
        if N_WARM:
            warm = wpp.tile([128, 512], F32, name="warm", tag="warm")
            wrow = ident[0:1, :]
            wmov = wrow.unsqueeze(1).broadcast_to([1, 4, 128])
            for _ in range(N_WARM):
                nc.tensor.matmul(warm[:], wrow, wmov, start=True, stop=True)

        # ---- single input DMA: host pre-interleaves pred|targ rows so
        # partition p holds pred rows 2p,2p+1 then targ rows 16p..16p+15.
        params = t([G * 5], "params")
        nc.sync.dma_start(out=params[:], in_=cat)

        P5 = params.rearrange("p (g c) -> p c g", c=5)  # [128, 5, 18]
        kap = P5[:, 3, :]  # [128, 18] stride-5 slabs
        bet = P5[:, 4, :]
        angles = P5[:, 0:3, :]  # eta, alpha, psi
        kap_p, kap_t = kap[:, 0:GP], kap[:, GP:G]
        bet_t = bet[:, GP:G]
        TT = slice(GP, G)

        # feature tiles (fp16); constant rows preset in the preamble shadow
        VF = t([K, GT], "VF", F16)
        nc.gpsimd.memset(VF[:, 0, :], 1.0)
        UF = t([K, GP], "UF", F16)
        nc.gpsimd.memset(UF[:, 1, :], 1.0)

        # tiles
        absv = t([3, G], "absv")
        trig = t([6, G], "trig")
        x2 = t([G], "x2")
        s4 = t([G], "s4")
        D = t([G], "D")
        LNOUT = t([G], "LNOUT")
        pp4 = t([2, 2, G], "pp4")
        m24 = t([2, G], "m24")
        mm4 = t([2, 2, G], "mm4")
        g23 = t([2, 3, GT], "g23")
        p23 = t([2, 6, GT], "p23")
        dV = t([6, GT], "dV")
        gam1 = t([3, G], "gam1")
        ppd = t([3, GP], "ppd")
        ppo = t([3, GP], "ppo")
        r = t([GP], "r")
        neg = t([GP], "neg")
        l1 = t([GP], "l1")
        kD = t([GP], "kD")
        Q = t([GP], "Q")
        r2 = t([GP], "r2")
        l2 = t([GP], "l2")
        dE = t([GP], "dE")
        edt = t([3, GP], "edt")
        kad = t([GP], "kad")
        a1 = t([GP], "a1")

        ce, ca, cp = trig[:, 0, :], trig[:, 1, :], trig[:, 2, :]
        se, sa, sp = trig[:, 3, :], trig[:, 4, :], trig[:, 5, :]
        T6 = trig.rearrange("p (s a) g -> p s a g", a=3)  # s: cos/sin, a: angle
        cpsp = T6[:, :, 2, :]  # rows {cp, sp}  [128, 2, 18]
        cese = T6[:, :, 0, :]  # rows {ce, se}  [128, 2, 18]
        m2ce, m2se = mm4[:, 0, 0, :], mm4[:, 0, 1, :]
        m4ce, m4se = mm4[:, 1, 0, :], mm4[:, 1, 1, :]
        cpce, cpse = pp4[:, 0, 0, :], pp4[:, 0, 1, :]
        spce, spse = pp4[:, 1, 0, :], pp4[:, 1, 1, :]
        g1p = gam1[:, :, 0:GP]  # [128, 3, 2]
        D_p = D[:, 0:GP]

        # =====================================================================
        # Feature computation.  IMPORTANT: the tile framework is sequentially
        # consistent with Python ISSUE order -- a read issued before the
        # producing write sees the OLD tile contents.  Everything below is in
        # topological order; engine choice (V=DVE, P=Pool, ACT) balances load.
        # =====================================================================
        V = nc.vector
        P = nc.gpsimd

        # |angles| feeds ACT cos; trig rows: 0..2 = cos(e,a,p), 3..5 = sin
        V.scalar_tensor_tensor(absv[:], angles, -1.0, angles, ALU.mult, ALU.max)
        nc.scalar.activation(trig[:, 3:6, :], angles, AF.Sin)
        nc.scalar.activation(trig[:, 0:3, :], absv[:], AF.Sin,
                             bias=half_pi, scale=-1.0)
        # c-denominator for all 18 groups; Ln table load overlaps DVE/Pool
        V.tensor_mul(x2, kap, kap)                                   # k^2
        V.scalar_tensor_tensor(s4, bet, 4.0, bet, ALU.mult, ALU.mult)  # 4b^2
        V.tensor_sub(D, x2, s4)                                      # k^2-4b^2
        nc.scalar.activation(LNOUT[:], D[:], AF.Ln, bias=eps_c)
        # trig products (Pool m24->mm4 chain runs parallel to DVE pp4)
        P.tensor_mul(m24, cpsp, ca.unsqueeze(1).broadcast_to([128, 2, G]))
        V.tensor_mul(
            pp4,
            cpsp.unsqueeze(2).broadcast_to([128, 2, 2, G]),
            cese.unsqueeze(1).broadcast_to([128, 2, 2, G]),
        )
        P.tensor_mul(
            mm4,
            m24.unsqueeze(2).broadcast_to([128, 2, 2, G]),
            cese.unsqueeze(1).broadcast_to([128, 2, 2, G]),
        )
        P.tensor_copy(gam1[:, 0, :], ca)
        P.tensor_mul(gam1[:, 1:3, :], cese,
                     sa.unsqueeze(1).broadcast_to([128, 2, G]))
        V.scalar_tensor_tensor(g23[:, 0, 0, :], cp[:, TT], -1.0, sa[:, TT],
                               ALU.mult, ALU.mult)                   # -cp*sa
        # pred lambda chain: l1 = (D-k)/D, l2 = (2k(D-k) - s)/(2D^2)
        V.reciprocal(r, D_p)
        V.tensor_sub(neg, D_p, kap_p)
        V.tensor_mul(l1, neg, r)
        V.tensor_mul(kD, kap_p, neg)                                 # k(D-k)
        V.scalar_tensor_tensor(Q, kD, 2.0, s4[:, 0:GP],
                               ALU.mult, ALU.subtract)               # 2k(D-k)-s
        V.tensor_mul(r2, r, r)
        V.scalar_tensor_tensor(l2, Q, 0.5, r2, ALU.mult, ALU.mult)
        V.tensor_sub(dE, l1, l2)
        # gamma2 / gamma3 components (targets only)
        P.tensor_sub(g23[:, 0, 1, :], m2ce[:, TT], spse[:, TT])
        P.tensor_add(g23[:, 0, 2, :], m2se[:, TT], spce[:, TT])
        P.tensor_mul(g23[:, 1, 0, :], sp[:, TT], sa[:, TT])
        V.scalar_tensor_tensor(g23[:, 1, 1, :], m4ce[:, TT], -1.0,
                               cpse[:, TT], ALU.mult, ALU.subtract)  # g3_1
        P.tensor_sub(g23[:, 1, 2, :], cpce[:, TT], m4se[:, TT])
        # pred g1 pair products: diag squares on ACT (its tables are already
        # loaded, so it is free after LNOUT), offdiags on Pool
        nc.scalar.activation(ppd[:], g1p, AF.Square)
        P.tensor_mul(
            ppo[:, 0:2, :],
            gam1[:, 0:1, 0:GP].broadcast_to([128, 2, GP]),
            gam1[:, 1:3, 0:GP],
        )
        P.tensor_mul(ppo[:, 2, :], gam1[:, 1, 0:GP], gam1[:, 2, 0:GP])
        # target pair products: squares on ACT, offdiags on Pool
        nc.scalar.activation(p23[:, :, 0:3, :], g23[:], AF.Square)
        P.tensor_mul(
            p23[:, :, 3:5, :],
            g23[:, :, 0:1, :].broadcast_to([128, 2, 2, GT]),
            g23[:, :, 1:3, :],
        )
        P.tensor_mul(p23[:, :, 5, :], g23[:, :, 1, :], g23[:, :, 2, :])
        V.tensor_sub(dV, p23[:, 1, :, :], p23[:, 0, :, :])
        # VF build (fp16 out): rank 1 = c_b' = k - 0.5 ln(prod)  (ln2pi
        # cancels against c_a); 2-4 = -k*g1; 5-7 = b*dV_d; 8-10 = 2b*dV_o
        V.scalar_tensor_tensor(VF[:, 1, :], LNOUT[:, GP:G], -0.5, kap_t,
                               ALU.mult, ALU.add)
        V.scalar_tensor_tensor(
            VF[:, 2:5, :], gam1[:, :, GP:G], -1.0,
            kap_t.unsqueeze(1).broadcast_to([128, 3, GT]),
            ALU.mult, ALU.mult,
        )
        P.tensor_mul(VF[:, 5:8, :], dV[:, 0:3, :],
                     bet_t.unsqueeze(1).broadcast_to([128, 3, GT]))
        V.scalar_tensor_tensor(
            VF[:, 8:11, :], dV[:, 3:6, :], 2.0,
            bet_t.unsqueeze(1).broadcast_to([128, 3, GT]),
            ALU.mult, ALU.mult,
        )
        # UF build
        l1b = l1.unsqueeze(1).broadcast_to([128, 3, GP])
        dEb = dE.unsqueeze(1).broadcast_to([128, 3, GP])
        V.tensor_mul(UF[:, 2:5, :], g1p, l1b)
        V.tensor_mul(edt, ppd, dEb)
        V.tensor_add(UF[:, 5:8, :], edt,
                     l2.unsqueeze(1).broadcast_to([128, 3, GP]))
        V.tensor_mul(UF[:, 8:11, :], ppo, dEb)
        V.tensor_mul(kad, kap_p, l1)
        V.scalar_tensor_tensor(a1, LNOUT[:, 0:GP], 0.5, kap_p,
                               ALU.mult, ALU.subtract)
        V.tensor_add(UF[:, 0, :], a1, kad)

        # =====================================================================
        # PE: transposes (fp16) + matmuls, interleaved so matmuls start as VT
        # chunks land.  UT first (it gates every matmul).
        # =====================================================================
        # pred rows are loaded blocked (row 128t + p on partition p), so the
        # transposed feature block for group t is directly UT cols 128t..+128
        UT = pool.tile([K, NS], F16, name="UT", tag="UT")
        utp = upp.tile([K, GP * 128], F16, name="utp", tag="utp")
        for j in range(GP):
            nc.tensor.transpose(utp[:, j * 128 : (j + 1) * 128], UF[:, :, j],
                                ident[:])
        nc.scalar.copy(UT[:], utp[:])

        # PSUM can only be read by the scalar (ACT) and vector (DVE) engines
        VT = pool.tile([K, M], F16, name="VT", tag="VT")
        copy_engines = [nc.scalar, nc.vector]

        def vt_chunk(q):
            # two half-copies on both PSUM-capable engines: halves the
            # transpose-to-matmul latency vs one [11, 512] copy
            vtp = tpp.tile([K, 512], F16, name="vtp", tag="vtp", bufs=2)
            for jj in range(4):
                j = q * 4 + jj
                nc.tensor.transpose(
                    vtp[:, jj * 128 : (jj + 1) * 128], VF[:, :, j], ident[:]
                )
            nc.scalar.copy(
                VT[:, q * 512 : q * 512 + 256], vtp[:, 0:256]
            )
            nc.vector.tensor_copy(
                VT[:, q * 512 + 256 : (q + 1) * 512], vtp[:, 256:512]
            )

        # main matmuls: targets are loaded blocked (row 128j + p on partition
        # p), so VT col 128j + p = target row and psum col f of chunk c is
        # target row 512c + f -- each matmul depends on exactly one vt_chunk
        mm_i = 0

        def mm(ti, c):
            nonlocal mm_i
            ops = opp.tile([128, 512], F32, name="ops", tag="ops")
            nc.tensor.matmul(
                ops[:],
                UT[:, 128 * ti : 128 * (ti + 1)],
                VT[:, 512 * c : 512 * (c + 1)],
                start=True,
                stop=True,
            )
            out_sb = pool.tile([128, 512], F32, name="out_sb", tag="out_sb",
                               bufs=4)
            orow = out[128 * ti : 128 * (ti + 1), 512 * c : 512 * (c + 1)]
            if mm_i == 0:
                # fast-path the first block: halve the PSUM->SBUF copy across
                # both engines and issue its DMA immediately -- the whole
                # output drain is BW-bound from the FIRST packet onward
                nc.scalar.copy(out_sb[:, 0:256], ops[:, 0:256])
                nc.sync.dma_start(out=orow[:, 0:256], in_=out_sb[:, 0:256])
                nc.vector.tensor_copy(out_sb[:, 256:512], ops[:, 256:512])
                nc.sync.dma_start(out=orow[:, 256:512], in_=out_sb[:, 256:512])
            else:
                eng = copy_engines[mm_i % 2]
                if eng is nc.scalar:
                    nc.scalar.copy(out_sb[:], ops[:])
                else:
                    eng.tensor_copy(out_sb[:], ops[:])
                nc.sync.dma_start(out=orow, in_=out_sb[:])
            mm_i += 1

        for c in range(4):
            vt_chunk(c)
            mm(0, c)
            mm(1, c)


def build():
    nc = bacc.Bacc()
    cat = nc.dram_tensor("cat", [128, G * 5], F32, kind="ExternalInput")
    out = nc.dram_tensor("out", [NS, M], F32, kind="ExternalOutput")
    with tile.TileContext(nc) as tc:
        _body(tc, cat[:], out[:])
    nc.finalize()
    return nc


_NC_CACHE = None


def _get_nc():
    global _NC_CACHE
    if _NC_CACHE is None:
        _NC_CACHE = build()
    return _NC_CACHE


def kernel(kent_pred, kent_target, trace=False, tmpdir=None):
    from concourse.bass_utils import run_bass_kernel_spmd

    nc = _get_nc()
    kent_pred = np.ascontiguousarray(np.asarray(kent_pred, dtype=np.float32))
    kent_target = np.ascontiguousarray(np.asarray(kent_target, dtype=np.float32))
    # blocked layout: partition p holds pred rows {p, 128+p} (local) then
    # targ rows {128j + p} -- one contiguous [128, 90] buffer per core
    targ_il = np.ascontiguousarray(
        kent_target.reshape(GT, 128, 5).transpose(1, 0, 2).reshape(128, GT * 5)
    )
    in_maps = []
    for i in range(NCORES):
        pred_il = (
            kent_pred[i * NS : (i + 1) * NS]
            .reshape(GP, 128, 5)
            .transpose(1, 0, 2)
            .reshape(128, GP * 5)
        )
        in_maps.append(
            {"cat": np.ascontiguousarray(np.concatenate([pred_il, targ_il], axis=1))}
        )
    res = run_bass_kernel_spmd(
        nc, in_maps, core_ids=list(range(NCORES)), trace=trace, tmpdir=tmpdir
    )
    out = np.concatenate([r["out"] for r in res.results], axis=0)
    if trace:
        kernel.last_results = res
    return out
